# revision 3
# baseline (speedup 1.0000x reference)
"""Per-edge dot-product (GNN DotProductPredictor) Bass kernel for 8 trn2 cores.

score[e] = sum_k h[src[e], k] * h[dst[e], k]

Strategy (SBUF-resident fp16 table + merged gpsimd ap_gather):
  - Edges sharded contiguously across 8 cores (400k each).
  - h cast to fp16, packed [128, 12500, 4]: partition p = 16s+q holds
    features [4q,4q+4) of node shard s. 12.8MB SBUF, loaded once.
  - Edges bucket by (src_shard s, dst_shard s2), group k = (s2-s)%8.
    Within a bucket, edges are sorted by src-local index (better Q7
    read-request locality). One merged ap_gather per chunk fetches
    C src rows AND C dst rows (num_idxs=2C=2048) -- the ~12us fixed
    instruction cost amortizes over 2048 idxs.
  - dst half is rotated up 16k partitions (SBUF->SBUF DMA), DVE multiply
    (in place), pair-reduce, block-diag matmul sums 16 partitions/shard.
  - Static plan: 8 groups x CAP_CH=6 chunks of C=1024 slots; capacity
    6144 sits ~1.4 sigma below the 6250 bucket mean, so ~2% of edges
    spill to an exact f32 numpy fallback on the host and device slots
    are ~fully utilized.
"""

import numpy as np

N_NODES = 100000
N_EDGES = 3200000
D = 64
N_CORES = 8

EPC = N_EDGES // N_CORES  # 400000

NSH = 8  # node shards
SHARD = N_NODES // NSH  # 12500 nodes per shard
FPP = D // 16  # 4 features per partition within a shard

C = 1024  # edge slots per chunk per core
CAP_CH = 6  # chunks per (core, group)
G = NSH * CAP_CH  # 48 chunks total
CAP = CAP_CH * C  # 6144 positions per (core, group)

_NC = None


def _build_nc(bufs=3, reps=1):
    import contextlib

    import concourse.bacc as bacc
    import concourse.tile as tile
    from concourse import mybir

    nc = bacc.Bacc("TRN2", target_bir_lowering=False)
    tab_d = nc.dram_tensor(
        "tab", [128, SHARD * FPP], mybir.dt.float16, kind="ExternalInput"
    )
    idx_d = nc.dram_tensor("idx", [G, 128, 2 * C // 16], mybir.dt.int16, kind="ExternalInput")
    bd_d = nc.dram_tensor("bd", [128, NSH], mybir.dt.float32, kind="ExternalInput")
    out_d = nc.dram_tensor("out", [G, NSH, C], mybir.dt.float32, kind="ExternalOutput")

    with tile.TileContext(nc) as tc:
        with tc.tile_pool(name="setup", bufs=1) as sp:
            tab = sp.tile([128, SHARD, FPP], mybir.dt.float16)
            nc.sync.dma_start(
                out=tab[:], in_=tab_d[:, :].rearrange("p (n j) -> p n j", j=FPP)
            )
            bd = sp.tile([128, NSH], mybir.dt.float32)
            nc.sync.dma_start(out=bd[:], in_=bd_d[:, :])
            loop = tc.For_i(0, reps, 1) if reps > 1 else contextlib.nullcontext()
            with loop, tc.tile_pool(name="pool", bufs=bufs) as pool, tc.tile_pool(
                name="psum", bufs=max(2, bufs), space="PSUM"
            ) as pp:
                for g in range(G):
                    k = g // CAP_CH
                    r = 16 * k
                    idxt = pool.tile([128, 2 * C // 16], mybir.dt.int16, tag="idxt")
                    nc.sync.dma_start(out=idxt[:], in_=idx_d[g, :, :])
                    gsd = pool.tile([128, 2 * C, FPP], mybir.dt.float16, tag="gsd")
                    nc.gpsimd.ap_gather(
                        gsd[:], tab[:], idxt[:],
                        channels=128, num_elems=SHARD, d=FPP, num_idxs=2 * C,
                    )
                    gs = gsd[:, :C, :]
                    gd = gsd[:, C:, :]
                    if k == 0:
                        rgd = gd
                    else:
                        rgdt = pool.tile([128, C, FPP], mybir.dt.float16, tag="rgd")
                        nc.sync.dma_start(out=rgdt[: 128 - r], in_=gd[r:])
                        nc.sync.dma_start(out=rgdt[128 - r :], in_=gd[:r])
                        rgd = rgdt[:]
                    nc.vector.tensor_tensor(
                        out=gs, in0=gs, in1=rgd, op=mybir.AluOpType.mult
                    )
                    red = pool.tile([128, C], mybir.dt.float32, tag="red")
                    nc.vector.tensor_reduce(
                        out=red[:], in_=gs,
                        axis=mybir.AxisListType.X, op=mybir.AluOpType.add,
                    )
                    ps = pp.tile([NSH, C], mybir.dt.float32, tag="ps")
                    nc.tensor.matmul(
                        ps[:, :512], bd[:], red[:, :512], start=True, stop=True
                    )
                    nc.tensor.matmul(
                        ps[:, 512:], bd[:], red[:, 512:], start=True, stop=True
                    )
                    score = pool.tile([NSH, C], mybir.dt.float32, tag="score")
                    nc.scalar.copy(out=score[:], in_=ps[:])
                    nc.sync.dma_start(out=out_d[g, :, :], in_=score[:])
    nc.compile()
    return nc


def _pack_table(h):
    h16 = h.astype(np.float16)
    # partition p = 16*s + q: features [4q, 4q+4) of nodes [12500s, ...)
    return np.ascontiguousarray(
        h16.reshape(NSH, SHARD, 16, FPP).transpose(0, 2, 1, 3).reshape(128, SHARD * FPP)
    )


def _prep_core(src_c, dst_c):
    """Build device idx tiles and the device-position map for one core.

    Returns (idx_dev [G, 128, 2*C//16] int16,
             edge_pos [EPC] int64 (-1 = overflow),
             overflow [EPC] bool)
    """
    s = (src_c // SHARD).astype(np.int64)
    s2 = (dst_c // SHARD).astype(np.int64)
    src_loc = src_c % SHARD
    dst_loc = dst_c % SHARD
    k = (s2 - s) % NSH
    key = k * NSH + s  # bucket id, group-major
    # sort by bucket, then by src-local index within the bucket
    order = np.argsort(key * SHARD + src_loc, kind="stable")
    counts = np.bincount(key, minlength=64)
    starts = np.concatenate([[0], np.cumsum(counts)])

    q_ord = np.arange(EPC, dtype=np.int64) - starts[key[order]]
    ok_ord = q_ord < CAP

    e = order[ok_ord]
    q = q_ord[ok_ord]
    ks = key[e] // NSH
    ss = key[e] % NSH
    g = ks * CAP_CH + q // C
    j = q % C

    IC = C // 16
    idx_dev = np.zeros((G, 128, 2 * IC), np.int16)
    idx_dev[g, 16 * ss + j % 16, j // 16] = src_loc[e].astype(np.int16)
    idx_dev[g, 16 * ((ss + ks) % NSH) + j % 16, IC + j // 16] = dst_loc[e].astype(
        np.int16
    )

    edge_pos = np.full(EPC, -1, np.int64)
    edge_pos[e] = (g * NSH + ss) * C + j
    overflow = np.zeros(EPC, bool)
    overflow[order[~ok_ord]] = True
    return idx_dev, edge_pos, overflow


def _blockdiag():
    bd = np.zeros((128, NSH), np.float32)
    for sh in range(NSH):
        bd[16 * sh : 16 * sh + 16, sh] = 1.0
    return bd


def kernel(h, src, dst, _trace=False):
    global _NC
    from concourse import bass_utils

    h = np.ascontiguousarray(np.asarray(h), dtype=np.float32)
    src = np.asarray(src).astype(np.int64)
    dst = np.asarray(dst).astype(np.int64)

    if _NC is None:
        _NC = _build_nc()

    tab = _pack_table(h)
    bd = _blockdiag()

    in_maps = []
    maps = []
    for c in range(N_CORES):
        lo = c * EPC
        idx_dev, edge_pos, overflow = _prep_core(src[lo : lo + EPC], dst[lo : lo + EPC])
        in_maps.append({"tab": tab, "idx": idx_dev, "bd": bd})
        maps.append((edge_pos, overflow))

    res = bass_utils.run_bass_kernel_spmd(
        _NC, in_maps, core_ids=list(range(N_CORES)), trace=_trace
    )

    out = np.empty(N_EDGES, np.float32)
    for c in range(N_CORES):
        lo = c * EPC
        edge_pos, overflow = maps[c]
        dev_out = res.results[c]["out"].reshape(-1)
        ok = ~overflow
        out[lo : lo + EPC][ok] = dev_out[edge_pos[ok]]
        if overflow.any():  # beyond static capacity: exact f32 host fallback
            e = np.nonzero(overflow)[0]
            ss = src[lo : lo + EPC][e]
            dd = dst[lo : lo + EPC][e]
            out[lo : lo + EPC][e] = np.einsum("ij,ij->i", h[ss], h[dd])
    out = out.reshape(N_EDGES, 1)
    if _trace:
        return out, res
    return out
